# revision 17
# baseline (speedup 1.0000x reference)
"""Trainium2 Bass kernel for nn_ContrastLoss (bidirectional NT-Xent-style loss).

Strategy (8 NeuronCores, SPMD):
  - Row-shard the N=8192 batch: core c owns rows r_c = [1024c, 1024(c+1)).
  - Every core redundantly computes the (tiny) projection z = W2@elu(W1@x+b1)+b2
    and the L2 row-normalization for BOTH views, in transposed layout
    [hid=64, n] so the big similarity matmuls need no further transposes.
    The core's own 1024 rows are appended as extra columns ("mine" block) so
    the SPMD program needs no core-dependent slicing: each core receives
    x?cat = [x.T | x.T[:, r_c]] with a ones row for the layer-1 bias.
    Matmul operands are bf16 (1 cycle/row on the PE); stats/accumulations stay
    fp32.
  - Main loop per core: for each 128-row tile of its shard and both stream
    directions (sc: n1[r]@n2.T, mp: n2[r]@n1.T), compute S in PSUM, E =
    exp(S/tau) on ACT with accumulated row sums, then a fused (E*pos)
    multiply+row-reduce (scalar_tensor_tensor) split between the DVE and
    GPSIMD engines against the streamed positive_matrix rows.
  - Finale per core: -log(masked/(rowsum+1e-8)+1e-8) summed over its rows for
    both streams -> single scalar partial. Host sums the 8 partials.
"""

import os

import ml_dtypes
import numpy as np

import concourse.bass as bass
import concourse.mybir as mybir
import concourse.tile as tile
from concourse import bacc
from concourse.bass_utils import run_bass_kernel_spmd

F32 = mybir.dt.float32
BF16 = mybir.dt.bfloat16
AF = mybir.ActivationFunctionType
ALU = mybir.AluOpType

N = 8192
HID = 64
M = 8              # cores
NM = N // M        # rows per core (1024)
NCAT = N + NM      # 9216 columns in the projected tensors (full | mine)
P = 128            # partitions
NT = NM // P       # row tiles per core (8)
CW = 2048          # main-loop chunk width
NCH = N // CW      # col chunks per row tile (4)
MVW = 512          # matmul moving width (PSUM bank limit: 512 fp32 out)
PJW = 1024         # projection chunk width
NPJ = NCAT // PJW  # projection chunks (9)
TAU = 0.8
LAMBDA = 0.5
EPS = 1e-8

# which masked-reduce chunks use the GPSIMD-multiply path (rest: DVE fused).
# gpsimd TensorTensor mult ~4.1us/chunk; DVE fused STT ~2.2us/chunk; the
# gpsimd path still needs a cheap (2x-mode) DVE accumulate pass ~1.2us.
GPS_FRAC = 0  # gpsimd TensorTensor corrupts results on real HW; keep all-DVE
def _use_gpsimd(idx: int) -> bool:
    return (idx * GPS_FRAC) % 64 < GPS_FRAC



def _build_body(tc, ins, out_ap):
    nc = tc.nc
    x1c, x2c, w1e, w2t, b2c, pos = (
        ins["x1cat"], ins["x2cat"], ins["w1e"], ins["w2t"], ins["b2c"], ins["pos"],
    )
    ds = bass.ds

    from contextlib import ExitStack

    with ExitStack() as ctx:
        const_pool = ctx.enter_context(tc.tile_pool(name="const", bufs=1))
        zn_pool = ctx.enter_context(tc.tile_pool(name="zn", bufs=1))
        strip_pool = ctx.enter_context(tc.tile_pool(name="strips", bufs=1))

        w1e_sb = const_pool.tile([65, HID], BF16, tag="w1e")
        nc.sync.dma_start(out=w1e_sb[:], in_=w1e[:])
        # W2.T duplicated at partition bases 0 and 64 (lhsT base must match rhs)
        w2x = const_pool.tile([P, HID], BF16, tag="w2x")
        nc.sync.dma_start(out=w2x[0:HID, :], in_=w2t[:])
        nc.sync.dma_start(out=w2x[HID:P, :], in_=w2t[:])
        b2c_sb = const_pool.tile([P, 1], F32, tag="b2c")
        nc.sync.dma_start(out=b2c_sb[:], in_=b2c[:])
        ones_col = const_pool.tile([P, 1], F32, tag="onesc")
        nc.vector.memset(ones_col[:], 1.0)
        # all-ones [128, 64] for column-sum broadcast matmuls
        ones_sq = const_pool.tile([P, HID], BF16, tag="onessq")
        nc.vector.memset(ones_sq[:], 1.0)

        # normalized projections, both views: rows 0:64 = z1n.T, 64:128 = z2n.T
        zn = zn_pool.tile([P, N], BF16, tag="zn")
        # mine block, swapped halves: rows 0:64 = z2n.T, 64:128 = z1n.T
        mine_sw = zn_pool.tile([P, NM], BF16, tag="minesw")

        # accumulator strips: per (row-tile, chunk) partials
        rs1 = strip_pool.tile([P, NT * NCH], F32, tag="rs1")
        rs2 = strip_pool.tile([P, NT * NCH], F32, tag="rs2")
        mk1 = strip_pool.tile([P, NT * NCH], F32, tag="mk1")
        mk2 = strip_pool.tile([P, NT * NCH], F32, tag="mk2")

        # ---------------- projection + normalization ----------------
        with ExitStack() as pctx:
            xc_pool = pctx.enter_context(tc.tile_pool(name="xc", bufs=1))
            helu_pool = pctx.enter_context(tc.tile_pool(name="helu", bufs=1))
            em_pool = pctx.enter_context(tc.tile_pool(name="em", bufs=3))
            zsq_pool = pctx.enter_context(tc.tile_pool(name="zsq", bufs=2))
            nrm_pool = pctx.enter_context(tc.tile_pool(name="nrm", bufs=2))
            pp = pctx.enter_context(tc.tile_pool(name="pp", bufs=3, space="PSUM"))
            pq = pctx.enter_context(tc.tile_pool(name="pq", bufs=1, space="PSUM"))

            x1_sb = xc_pool.tile([65, NCAT], BF16, tag="x1c")
            nc.sync.dma_start(out=x1_sb[:], in_=x1c[:])
            x2_sb = xc_pool.tile([65, NCAT], BF16, tag="x2c")
            nc.sync.dma_start(out=x2_sb[:], in_=x2c[:])

            helu = helu_pool.tile([P, NCAT], BF16, tag="helu")

            # ---- layer 1 + ELU:  helu = elu(W1 @ x.T + b1) for both views
            for c in range(NPJ):
                hp = pp.tile([P, PJW], F32, tag="pp")
                for q in range(PJW // MVW):
                    sl = ds(c * PJW + q * MVW, MVW)
                    qs = ds(q * MVW, MVW)
                    nc.tensor.matmul(hp[0:HID, qs], w1e_sb[:], x1_sb[:, sl],
                                     start=True, stop=True)
                    nc.tensor.matmul(hp[HID:P, qs], w1e_sb[:], x2_sb[:, sl],
                                     start=True, stop=True)
                sl = ds(c * PJW, PJW)
                e_t = em_pool.tile([P, PJW], F32, tag="em")
                nc.scalar.activation(e_t[:], hp[:], AF.Exp)
                m_t = em_pool.tile([P, PJW], F32, tag="em")
                nc.vector.tensor_scalar(m_t[:], e_t[:], 1.0, -1.0, op0=ALU.min,
                                        op1=ALU.add)
                # elu(h) = max(h, min(exp(h),1)-1)
                nc.vector.tensor_tensor(helu[:, sl], hp[:], m_t[:], op=ALU.max)

            # ---- layer 2 + normalize, fully chunk-local:
            # zp = W2 @ helu (+swap halves for the mine chunk); ssq broadcast
            # via ones-matmul; inv = 1/max(sqrt(ssq),eps); zn = (zp+b2)*inv
            for c in range(NPJ):
                is_mine = c * PJW >= N
                zp = pp.tile([P, PJW], F32, tag="pp")
                sb = pp.tile([P, PJW], F32, tag="pp")
                if not is_mine:
                    d1, d2 = slice(0, HID), slice(HID, P)
                else:  # swap output halves for the mine block
                    d1, d2 = slice(HID, P), slice(0, HID)
                for q in range(PJW // MVW):
                    sl = ds(c * PJW + q * MVW, MVW)
                    qs = ds(q * MVW, MVW)
                    nc.tensor.matmul(zp[d1, qs], w2x[0:HID, :],
                                     helu[0:HID, sl], start=True, stop=True)
                    nc.tensor.matmul(zp[d2, qs], w2x[HID:P, :],
                                     helu[HID:P, sl], start=True, stop=True)
                zq = zsq_pool.tile([P, PJW], BF16, tag="zsq")
                nc.scalar.activation(zq[:], zp[:], AF.Square)
                for q in range(PJW // MVW):
                    qs = ds(q * MVW, MVW)
                    nc.tensor.matmul(sb[0:HID, qs], ones_sq[0:HID, :],
                                     zq[0:HID, qs], start=True, stop=True)
                    nc.tensor.matmul(sb[HID:P, qs], ones_sq[HID:P, :],
                                     zq[HID:P, qs], start=True, stop=True)
                nrm_t = nrm_pool.tile([P, PJW], F32, tag="nrm")
                nc.scalar.activation(nrm_t[:], sb[:], AF.Sqrt)
                dmp = nrm_pool.tile([P, PJW], F32, tag="dmp")
                nc.vector.tensor_scalar(dmp[:], nrm_t[:], 1e-12, None,
                                        op0=ALU.max)
                invb = nrm_pool.tile([P, PJW], F32, tag="invb")
                nc.vector.reciprocal(invb[:], dmp[:])
                zb = zsq_pool.tile([P, PJW], F32, tag="zb")
                nc.scalar.activation(zb[:], zp[:], AF.Identity, bias=b2c_sb[:])
                dst = (zn[:, ds(c * PJW, PJW)] if not is_mine
                       else mine_sw[:, ds(c * PJW - N, PJW)])
                nc.vector.tensor_tensor(dst, zb[:], invb[:], op=ALU.mult)

        if os.environ.get("K_STAGE") == "proj":
            with ExitStack() as fctx:
                fin_pool = fctx.enter_context(tc.tile_pool(name="fin", bufs=1))
                dbg = fin_pool.tile([1, 1], F32, tag="dbg")
                nc.scalar.copy(dbg[:], mine_sw[0:1, 0:1])
                nc.sync.dma_start(out=out_ap[:], in_=dbg[:])
            return

        # ---------------- main similarity loop ----------------
        with ExitStack() as mctx:
            pos_pool = mctx.enter_context(tc.tile_pool(name="pos", bufs=6))
            e_pool = mctx.enter_context(tc.tile_pool(name="et", bufs=4))
            scr_pool = mctx.enter_context(tc.tile_pool(name="scr", bufs=4))
            pm = mctx.enter_context(tc.tile_pool(name="pm", bufs=2, space="PSUM"))

            idx = 0
            for t in range(NT):
                pos_tiles = []
                for k in range(NCH):
                    pt = pos_pool.tile([P, CW], F32, tag="pos")
                    nc.sync.dma_start(
                        out=pt[:], in_=pos[ds(t * P, P), ds(k * CW, CW)])
                    pos_tiles.append(pt)
                for s in range(2):  # 0: sc stream (n1[r]@n2.T), 1: mp stream
                    # stream A: lhsT = z1n-mine (base 64), rhs = z2n (base 64)
                    # stream B: lhsT = z2n-mine (base 0),  rhs = z1n (base 0)
                    lhsT = (mine_sw[HID:P, ds(t * P, P)] if s == 0
                            else mine_sw[0:HID, ds(t * P, P)])
                    rh = zn[HID:P, :] if s == 0 else zn[0:HID, :]
                    rs_strip = rs1 if s == 0 else rs2
                    mk_strip = mk1 if s == 0 else mk2
                    for k in range(NCH):
                        ps = pm.tile([P, CW], F32, tag="pm")
                        for q in range(CW // MVW):
                            nc.tensor.matmul(
                                ps[:, ds(q * MVW, MVW)], lhsT,
                                rh[:, ds(k * CW + q * MVW, MVW)],
                                start=True, stop=True)
                        e_t = e_pool.tile([P, CW], F32, tag="et")
                        col = ds(t * NCH + k, 1)
                        nc.scalar.activation(e_t[:], ps[:], AF.Exp,
                                             scale=float(1.0 / TAU),
                                             accum_out=rs_strip[:, col])
                        scr = scr_pool.tile([P, CW], F32, tag="scr")
                        if _use_gpsimd(idx):
                            # gpsimd multiplies, DVE reduces in 2x mode
                            nc.gpsimd.tensor_tensor(
                                scr[:], e_t[:], pos_tiles[k][:], op=ALU.mult)
                            sc2 = scr_pool.tile([P, CW], F32, tag="scr2")
                            nc.vector.tensor_scalar(
                                sc2[:], scr[:], 0.0, 1.0, op0=ALU.add,
                                op1=ALU.mult, accum_out=mk_strip[:, col])
                        else:
                            nc.vector.scalar_tensor_tensor(
                                out=scr[:], in0=e_t[:], scalar=1.0,
                                in1=pos_tiles[k][:], op0=ALU.mult, op1=ALU.mult,
                                accum_out=mk_strip[:, col])
                        idx += 1

        if os.environ.get("K_STAGE") == "main":
            with ExitStack() as fctx:
                fin_pool = fctx.enter_context(tc.tile_pool(name="fin", bufs=1))
                dbg = fin_pool.tile([1, 1], F32, tag="dbg")
                nc.scalar.copy(dbg[:], mk1[0:1, 0:1])
                nc.sync.dma_start(out=out_ap[:], in_=dbg[:])
            return

        # ---------------- finale ----------------
        with ExitStack() as fctx:
            fin_pool = fctx.enter_context(tc.tile_pool(name="fin", bufs=1))
            pf = fctx.enter_context(tc.tile_pool(name="pf", bufs=1, space="PSUM"))

            lnin = fin_pool.tile([P, 2 * NT], F32, tag="lnin")
            lnout = fin_pool.tile([P, 2 * NT], F32, tag="lnout")
            for s in range(2):
                rs_strip = rs1 if s == 0 else rs2
                mk_strip = mk1 if s == 0 else mk2
                rsf = fin_pool.tile([P, NT], F32, tag=f"rsf{s}")
                mkf = fin_pool.tile([P, NT], F32, tag=f"mkf{s}")
                nc.vector.tensor_reduce(
                    rsf[:], rs_strip[:].rearrange("p (t k) -> p t k", k=NCH),
                    axis=mybir.AxisListType.X, op=ALU.add)
                nc.vector.tensor_reduce(
                    mkf[:], mk_strip[:].rearrange("p (t k) -> p t k", k=NCH),
                    axis=mybir.AxisListType.X, op=ALU.add)
                den = fin_pool.tile([P, NT], F32, tag=f"den{s}")
                nc.vector.tensor_scalar(den[:], rsf[:], float(EPS), None,
                                        op0=ALU.add)
                rec = fin_pool.tile([P, NT], F32, tag=f"rec{s}")
                nc.vector.reciprocal(rec[:], den[:])
                nc.vector.tensor_tensor(lnin[:, ds(s * NT, NT)], mkf[:],
                                        rec[:], op=ALU.mult)
            lnacc = fin_pool.tile([P, 1], F32, tag="lnacc")
            epsb = fin_pool.tile([P, 1], F32, tag="epsb")
            nc.vector.memset(epsb[:], float(EPS))
            nc.scalar.activation(lnout[:], lnin[:], AF.Ln, bias=epsb[:],
                                 accum_out=lnacc[:])
            ps1 = pf.tile([1, 1], F32, tag="pf")
            nc.tensor.matmul(ps1[:], ones_col[:], lnacc[:], start=True, stop=True)
            res = fin_pool.tile([1, 1], F32, tag="res")
            # mean over N rows, x0.5 per stream, negate
            nc.scalar.activation(res[:], ps1[:], AF.Copy,
                                 scale=float(-LAMBDA / N))
            nc.sync.dma_start(out=out_ap[:], in_=res[:])


_CACHE = {}


def _build_program():
    if "nc" in _CACHE:
        return _CACHE["nc"], _CACHE["ins"], _CACHE["out"]
    nc = bacc.Bacc("TRN2", target_bir_lowering=False, debug=False,
                   num_devices=M)
    ins = {
        "x1cat": nc.dram_tensor("x1cat", [65, NCAT], BF16, kind="ExternalInput").ap(),
        "x2cat": nc.dram_tensor("x2cat", [65, NCAT], BF16, kind="ExternalInput").ap(),
        "w1e": nc.dram_tensor("w1e", [65, HID], BF16, kind="ExternalInput").ap(),
        "w2t": nc.dram_tensor("w2t", [HID, HID], BF16, kind="ExternalInput").ap(),
        "b2c": nc.dram_tensor("b2c", [P, 1], F32, kind="ExternalInput").ap(),
        "pos": nc.dram_tensor("pos", [NM, N], F32, kind="ExternalInput").ap(),
    }
    out_ap = nc.dram_tensor("out", [1, 1], F32, kind="ExternalOutput").ap()
    with tile.TileContext(nc) as tc:
        _build_body(tc, ins, out_ap)
    nc.compile()
    _CACHE["nc"] = nc
    _CACHE["ins"] = ins
    _CACHE["out"] = out_ap
    return nc, ins, out_ap


def _host_prep(x1, x2, W1, b1, W2, b2, positive_matrix):
    f32 = np.float32
    bf = ml_dtypes.bfloat16
    x1t = np.asarray(x1, f32).T
    x2t = np.asarray(x2, f32).T
    ones = np.ones((1, N), f32)
    w1e = np.ascontiguousarray(np.concatenate(
        [np.asarray(W1, f32).T, np.asarray(b1, f32)[None, :]], axis=0
    ).astype(bf))
    base1 = np.concatenate([x1t, ones], axis=0).astype(bf)   # [65, N]
    base2 = np.concatenate([x2t, ones], axis=0).astype(bf)
    w2t = np.ascontiguousarray(np.asarray(W2, f32).T.astype(bf))
    b2c = np.concatenate([np.asarray(b2, f32)] * 2)[:, None].copy()
    pos = np.ascontiguousarray(positive_matrix, dtype=f32)
    in_maps = []
    for c in range(M):
        rc = slice(c * NM, (c + 1) * NM)
        in_maps.append({
            "x1cat": np.ascontiguousarray(
                np.concatenate([base1, base1[:, rc]], axis=1)),
            "x2cat": np.ascontiguousarray(
                np.concatenate([base2, base2[:, rc]], axis=1)),
            "w1e": w1e,
            "w2t": w2t,
            "b2c": b2c,
            "pos": np.ascontiguousarray(pos[rc]),
        })
    return in_maps


def run_on_hw(in_maps, trace=False, **kw):
    nc, _, _ = _build_program()
    return run_bass_kernel_spmd(nc, in_maps, list(range(M)), trace=trace, **kw)


def kernel(x1, x2, W1, b1, W2, b2, positive_matrix):
    in_maps = _host_prep(x1, x2, W1, b1, W2, b2, positive_matrix)
    res = run_on_hw(in_maps)
    total = np.float32(0.0)
    for c in range(M):
        total += np.float32(res.results[c]["out"][0, 0])
    return np.float32(total)
